# revision 9
# baseline (speedup 1.0000x reference)
"""Trainium2 Bass kernel for the quirky-reshape 16-head attention layer.

Shapes (hardcoded): x [2, 2048, 1024], Wq/Wk/Wv/Wo [1024, 1024], n_head=16.

Sharding: core c in [0,8) handles batch b=c//4 and head group g=c%4 (heads
4g..4g+3). The reference's quirky `qkv.reshape(b, s, d)` merge makes output
rows [h*128, (h+1)*128) depend only on head h, so each core produces the
disjoint output row block [g*512, (g+1)*512) of its batch — no collectives.

Precision: q/k path (projections + scores) in fp16 (11-bit mantissa), exp /
AV / O-projection in bf16 (fp32-range needed: exp values reach ~1e30), all
matmul accumulation in fp32 PSUM. Measured end-to-end scale-relative absmax
error ~4e-3.

Per-core dataflow:
  qT/kT = W^T @ x^T            [256, 2048] fp16 pair tiles (2 heads x 64)
  V[kb] = per-head [ones | v]  [128, 4, 128] bf16 per 128-key block
  per (head, 1024-wide q chunk):
    for kb in 16: S^T[kb] = kT_h[:,kb].T @ qT_h  -> PSUM [128, 1024] fp32
                  E[kb] = exp(S^T[kb])           (ScalarE -> SBUF bf16)
                  AV[half] += [1|v].T @ E[kb]    -> PSUM [128, 512] x2
    rows 0:64 of AV = softmax denominator (broadcast), rows 64:128 = qkv.
    rcp = reciprocal_approx_fast(denom); DMA rcp to partitions 64:128;
    Q2[64:128, h, q] = qkv * rcp (bf16)
  Q2[0:64, h, 1:] = Q2[64:128, h, :-1]  (DMA shift: O-proj contraction layout)
  out^T[mb] = sum_kt Wo[kt,mb].T @ Q2[:, (h,c) flat][:, (2kt+1)::16]
"""

import numpy as np

B, S, D, H = 2, 2048, 1024, 16
DH = 64
NCORES = 8

_CACHE = {}


def _build_program():
    from concourse import bacc, tile, mybir

    F32 = mybir.dt.float32
    F16 = mybir.dt.float16
    BF16 = mybir.dt.bfloat16
    EXP = mybir.ActivationFunctionType.Exp

    nc = bacc.Bacc(None, target_bir_lowering=False, debug=False)

    xt_d = nc.dram_tensor("xt", [8, 128, 2048], F16, kind="ExternalInput").ap()
    wq_d = nc.dram_tensor("wq", [8, 128, 256], F16, kind="ExternalInput").ap()
    wk_d = nc.dram_tensor("wk", [8, 128, 256], F16, kind="ExternalInput").ap()
    wv_d = nc.dram_tensor("wv", [8, 128, 256], F16, kind="ExternalInput").ap()
    wo_d = nc.dram_tensor("wo", [8, 128, 1024], BF16, kind="ExternalInput").ap()
    out_d = nc.dram_tensor("out", [8, 128, 512], F32, kind="ExternalOutput").ap()

    with tile.TileContext(nc) as tc:
        with (
            tc.tile_pool(name="keep", bufs=1) as keep,
            tc.tile_pool(name="exp", bufs=6) as expp,
            tc.tile_pool(name="rcp", bufs=3) as rcpp,
            tc.tile_pool(name="osb", bufs=2) as osbp,
        ):
            # ---------------- phase 1: projections ----------------
            qk_sb = {}
            v_sb = {}
            with (
                tc.tile_pool(name="inp", bufs=1) as inp,
                tc.tile_pool(name="ps1", bufs=1, space="PSUM") as ps1,
            ):
                xt = []
                wq = []
                wk = []
                wv = []
                for kt in range(8):
                    t = inp.tile([128, 2048], F16, tag=f"xt{kt}", name=f"xt{kt}")
                    nc.sync.dma_start(out=t[:], in_=xt_d[kt])
                    xt.append(t)
                    for nm, lst, dram in (("wq", wq, wq_d), ("wk", wk, wk_d), ("wv", wv, wv_d)):
                        t = inp.tile([128, 256], F16, tag=f"{nm}{kt}", name=f"{nm}{kt}")
                        nc.sync.dma_start(out=t[:], in_=dram[kt])
                        lst.append(t)

                # qT / kT pair tiles [128, 2048]; pair p holds heads (2p, 2p+1)
                for nm, wt in (("q", wq), ("k", wk)):
                    for pair in range(2):
                        dst = keep.tile([128, 2048], F16, tag=f"{nm}T{pair}", name=f"{nm}T{pair}")
                        qk_sb[(nm, pair)] = dst
                        for ch in range(4):
                            ps = ps1.tile([128, 512], F32, tag="qk", bufs=2, name="qkps")
                            for kt in range(8):
                                nc.tensor.matmul(
                                    ps[:],
                                    wt[kt][:, pair * 128:(pair + 1) * 128],
                                    xt[kt][:, ch * 512:(ch + 1) * 512],
                                    start=(kt == 0),
                                    stop=(kt == 7),
                                )
                            nc.vector.tensor_copy(dst[:, ch * 512:(ch + 1) * 512], ps[:])

                # V tiles: [128, 4, 128]; head block hg = [ones(64) | v_hg(64)]
                for kb in range(16):
                    vt = keep.tile([128, 4, 128], BF16, tag=f"v{kb}", name=f"v{kb}")
                    v_sb[kb] = vt
                    nc.vector.memset(vt[:], 1.0)
                    ps = ps1.tile([128, 256], F32, tag="vp", bufs=2, name="vps")
                    for kt in range(8):
                        nc.tensor.matmul(
                            ps[:],
                            xt[kt][:, kb * 128:(kb + 1) * 128],
                            wv[kt][:],
                            start=(kt == 0),
                            stop=(kt == 7),
                        )
                    nc.vector.tensor_copy(vt[:, :, 64:128],
                                          ps[:].rearrange("p (a b) -> p a b", a=4))

            # ---------------- phase 2: attention ----------------
            late_cm = tc.tile_pool(name="late", bufs=1)
            late = late_cm.__enter__()
            q2 = late.tile([128, 4, 2048], BF16, tag="q2")
            wo = []
            for kt in range(8):
                t = late.tile([128, 1024], BF16, tag=f"wo{kt}", name=f"wo{kt}")
                nc.sync.dma_start(out=t[:], in_=wo_d[kt])
                wo.append(t)

            with tc.tile_pool(name="ps2", bufs=1, space="PSUM") as ps2:
                for pair in range(2):
                    qT = qk_sb[("q", pair)]
                    kT = qk_sb[("k", pair)]
                    for qc in range(2):
                        av = {}
                        for hl in range(2):
                            for half in range(2):
                                av[(hl, half)] = ps2.tile([128, 512], F32, tag="av", bufs=4, name="av")
                        for kb in range(16):
                            for hl in range(2):
                                hg = 2 * pair + hl
                                rows = slice(64 * hl, 64 * hl + 64)
                                sc = ps2.tile([128, 1024], F32, tag="sc", bufs=2, name="sc")
                                for sub in range(2):
                                    q0 = qc * 1024 + sub * 512
                                    nc.tensor.matmul(
                                        sc[:, sub * 512:(sub + 1) * 512],
                                        kT[rows, kb * 128:(kb + 1) * 128],
                                        qT[rows, q0:q0 + 512],
                                        start=True,
                                        stop=True,
                                    )
                                et = expp.tile([128, 1024], BF16, tag="exp", name="et")
                                nc.scalar.activation(et[:], sc[:], EXP)
                                lhsT = v_sb[kb][:, hg, :]
                                for half in range(2):
                                    nc.tensor.matmul(
                                        av[(hl, half)][:],
                                        lhsT,
                                        et[:, half * 512:(half + 1) * 512],
                                        start=(kb == 0),
                                        stop=(kb == 15),
                                    )
                        for hl in range(2):
                            hg = 2 * pair + hl
                            for half in range(2):
                                ap = av[(hl, half)]
                                rt = rcpp.tile([128, 512], F32, tag="rcp", name="rt")
                                nc.vector.reciprocal_approx_fast(rt[0:64, :], ap[0:64, :])
                                nc.sync.dma_start(out=rt[64:128, :], in_=rt[0:64, :])
                                q0 = qc * 1024 + half * 512
                                nc.vector.tensor_mul(
                                    q2[64:128, hg, q0:q0 + 512],
                                    ap[64:128, :],
                                    rt[64:128, :],
                                )
                    # shift-copy the lower 64 partitions for this pair's heads
                    for hl in range(2):
                        hg = 2 * pair + hl
                        nc.sync.dma_start(
                            out=q2[0:64, hg, 1:2048], in_=q2[64:128, hg, 0:2047]
                        )

            # ---------------- phase 3: output projection ----------------
            with tc.tile_pool(name="ps3", bufs=1, space="PSUM") as ps3:
                q2f = q2[:].rearrange("p h c -> p (h c)")
                for mb in range(8):
                    ops = ps3.tile([128, 512], F32, tag="o", bufs=2, name="ops")
                    for kt in range(8):
                        nc.tensor.matmul(
                            ops[:],
                            wo[kt][:, mb * 128:(mb + 1) * 128],
                            q2f[:, (2 * kt + 1)::16],
                            start=(kt == 0),
                            stop=(kt == 7),
                        )
                    ot = osbp.tile([128, 512], F32, tag="ot", name="ot")
                    nc.vector.tensor_copy(ot[:], ops[:])
                    nc.sync.dma_start(out=out_d[mb], in_=ot[:])
            late_cm.__exit__(None, None, None)

    nc.compile()
    return nc


def _get_program():
    if "nc" not in _CACHE:
        _CACHE["nc"] = _build_program()
    return _CACHE["nc"]


def _make_in_maps(x, Wq, Wk, Wv, Wo):
    import ml_dtypes

    bf16 = ml_dtypes.bfloat16
    wo8 = np.ascontiguousarray(Wo.astype(bf16)).reshape(8, 128, 1024)
    xts = [
        np.ascontiguousarray(x[b].T.astype(np.float16)).reshape(8, 128, 2048)
        for b in range(B)
    ]
    wq16 = Wq.astype(np.float16)
    wk16 = Wk.astype(np.float16)
    wv16 = Wv.astype(np.float16)
    in_maps = []
    for c in range(NCORES):
        b, g = c // 4, c % 4
        cols = slice(4 * g * DH, 4 * (g + 1) * DH)
        in_maps.append(
            {
                "xt": xts[b],
                "wq": np.ascontiguousarray(wq16[:, cols]).reshape(8, 128, 256),
                "wk": np.ascontiguousarray(wk16[:, cols]).reshape(8, 128, 256),
                "wv": np.ascontiguousarray(wv16[:, cols]).reshape(8, 128, 256),
                "wo": wo8,
            }
        )
    return in_maps


def kernel(x, Wq, Wk, Wv, Wo, n_head):
    from concourse.bass_utils import run_bass_kernel_spmd

    assert int(n_head) == H
    x = np.asarray(x, np.float32)
    Wq = np.asarray(Wq, np.float32)
    Wk = np.asarray(Wk, np.float32)
    Wv = np.asarray(Wv, np.float32)
    Wo = np.asarray(Wo, np.float32)

    nc = _get_program()
    in_maps = _make_in_maps(x, Wq, Wk, Wv, Wo)
    res = run_bass_kernel_spmd(nc, in_maps, list(range(NCORES)))

    out = np.empty((B, S, D), np.float32)
    for c in range(NCORES):
        b, g = c // 4, c % 4
        out[b, g * 512:(g + 1) * 512, :] = res.results[c]["out"].reshape(1024, 512).T
    return out


# revision 10
# speedup vs baseline: 1.0919x; 1.0919x over previous
"""Trainium2 Bass kernel for the quirky-reshape 16-head attention layer.

Shapes (hardcoded): x [2, 2048, 1024], Wq/Wk/Wv/Wo [1024, 1024], n_head=16.

Sharding: core c in [0,8) handles batch b=c//4 and head group g=c%4 (heads
4g..4g+3). The reference's quirky `qkv.reshape(b, s, d)` merge makes output
rows [h*128, (h+1)*128) depend only on head h, so each core produces the
disjoint output row block [g*512, (g+1)*512) of its batch — no collectives.

Precision: q/k path (projections + scores) in fp16 (11-bit mantissa), exp /
AV / O-projection in bf16 (fp32-range needed: exp values reach ~1e30), all
matmul accumulation in fp32 PSUM. Measured end-to-end scale-relative absmax
error ~4e-3.

Per-core dataflow:
  qT/kT = W^T @ x^T            [256, 2048] fp16 pair tiles (2 heads x 64)
  V[kb] = per-head [ones | v]  [128, 4, 128] bf16 per 128-key block
  per (head, 1024-wide q chunk):
    for kb in 16: S^T[kb] = kT_h[:,kb].T @ qT_h  -> PSUM [128, 1024] fp32
                  E[kb] = exp(S^T[kb])           (ScalarE -> SBUF bf16)
                  AV[half] += [1|v].T @ E[kb]    -> PSUM [128, 512] x2
    rows 0:64 of AV = softmax denominator (broadcast), rows 64:128 = qkv.
    rcp = reciprocal_approx_fast(denom); DMA rcp to partitions 64:128;
    Q2[64:128, h, q] = qkv * rcp (bf16)
  Q2[0:64, h, 1:] = Q2[64:128, h, :-1]  (DMA shift: O-proj contraction layout)
  out^T[mb] = sum_kt Wo[kt,mb].T @ Q2[:, (h,c) flat][:, (2kt+1)::16]
"""

import numpy as np

B, S, D, H = 2, 2048, 1024, 16
DH = 64
NCORES = 8

_CACHE = {}


def _build_program():
    from concourse import bacc, tile, mybir

    F32 = mybir.dt.float32
    F16 = mybir.dt.float16
    BF16 = mybir.dt.bfloat16
    EXP = mybir.ActivationFunctionType.Exp

    nc = bacc.Bacc(None, target_bir_lowering=False, debug=False)

    xt_d = nc.dram_tensor("xt", [8, 128, 2048], F16, kind="ExternalInput").ap()
    wq_d = nc.dram_tensor("wq", [8, 128, 256], F16, kind="ExternalInput").ap()
    wk_d = nc.dram_tensor("wk", [8, 128, 256], F16, kind="ExternalInput").ap()
    wv_d = nc.dram_tensor("wv", [8, 128, 256], F16, kind="ExternalInput").ap()
    wo_d = nc.dram_tensor("wo", [8, 128, 1024], BF16, kind="ExternalInput").ap()
    out_d = nc.dram_tensor("out", [8, 128, 512], F32, kind="ExternalOutput").ap()

    with tile.TileContext(nc) as tc:
        with (
            tc.tile_pool(name="keep", bufs=1) as keep,
            tc.tile_pool(name="exp", bufs=6) as expp,
            tc.tile_pool(name="rcp", bufs=3) as rcpp,
            tc.tile_pool(name="osb", bufs=2) as osbp,
        ):
            # ---------------- phase 1: projections ----------------
            qk_sb = {}
            v_sb = {}
            with (
                tc.tile_pool(name="inp", bufs=1) as inp,
                tc.tile_pool(name="ps1", bufs=1, space="PSUM") as ps1,
            ):
                xt = []
                wq = []
                wk = []
                wv = []
                for kt in range(8):
                    t = inp.tile([128, 2048], F16, tag=f"xt{kt}", name=f"xt{kt}")
                    nc.sync.dma_start(out=t[:], in_=xt_d[kt])
                    xt.append(t)
                    for nm, lst, dram in (("wq", wq, wq_d), ("wk", wk, wk_d), ("wv", wv, wv_d)):
                        t = inp.tile([128, 256], F16, tag=f"{nm}{kt}", name=f"{nm}{kt}")
                        nc.sync.dma_start(out=t[:], in_=dram[kt])
                        lst.append(t)

                # qT / kT pair tiles [128, 2048]; pair p holds heads (2p, 2p+1)
                for nm, wt in (("q", wq), ("k", wk)):
                    for pair in range(2):
                        dst = keep.tile([128, 2048], F16, tag=f"{nm}T{pair}", name=f"{nm}T{pair}")
                        qk_sb[(nm, pair)] = dst
                        for ch in range(4):
                            ps = ps1.tile([128, 512], F32, tag="qk", bufs=2, name="qkps")
                            for kt in range(8):
                                nc.tensor.matmul(
                                    ps[:],
                                    wt[kt][:, pair * 128:(pair + 1) * 128],
                                    xt[kt][:, ch * 512:(ch + 1) * 512],
                                    start=(kt == 0),
                                    stop=(kt == 7),
                                )
                            nc.vector.tensor_copy(dst[:, ch * 512:(ch + 1) * 512], ps[:])

                # V tiles: [128, 4, 128]; head block hg = [ones(64) | v_hg(64)]
                for kb in range(16):
                    vt = keep.tile([128, 4, 128], BF16, tag=f"v{kb}", name=f"v{kb}")
                    v_sb[kb] = vt
                    nc.vector.memset(vt[:], 1.0)
                    ps = ps1.tile([128, 256], F32, tag="vp", bufs=2, name="vps")
                    for kt in range(8):
                        nc.tensor.matmul(
                            ps[:],
                            xt[kt][:, kb * 128:(kb + 1) * 128],
                            wv[kt][:],
                            start=(kt == 0),
                            stop=(kt == 7),
                        )
                    nc.vector.tensor_copy(vt[:, :, 64:128],
                                          ps[:].rearrange("p (a b) -> p a b", a=4))

            # ---------------- phase 2: attention ----------------
            late_cm = tc.tile_pool(name="late", bufs=1)
            late = late_cm.__enter__()
            q2 = late.tile([128, 16, 4, 128], BF16, tag="q2")
            wo = []
            for kt in range(8):
                t = late.tile([128, 1024], BF16, tag=f"wo{kt}", name=f"wo{kt}")
                nc.sync.dma_start(out=t[:], in_=wo_d[kt])
                wo.append(t)

            with tc.tile_pool(name="ps2", bufs=1, space="PSUM") as ps2:
                for pair in range(2):
                    qT = qk_sb[("q", pair)]
                    kT = qk_sb[("k", pair)]
                    for qc in range(2):
                        av = {}
                        for hl in range(2):
                            for half in range(2):
                                av[(hl, half)] = ps2.tile([128, 512], F32, tag="av", bufs=4, name="av")
                        for kb in range(16):
                            sc = {}
                            for hl in range(2):
                                sc[hl] = ps2.tile([128, 1024], F32, tag="sc", bufs=2, name="sc")
                            # paired scores: head A (rows 0:64) and head B
                            # (rows 64:128) adjacent -> disjoint PE row groups
                            for sub in range(2):
                                q0 = qc * 1024 + sub * 512
                                for hl in range(2):
                                    rows = slice(64 * hl, 64 * hl + 64)
                                    nc.tensor.matmul(
                                        sc[hl][:, sub * 512:(sub + 1) * 512],
                                        kT[rows, kb * 128:(kb + 1) * 128],
                                        qT[rows, q0:q0 + 512],
                                        start=True,
                                        stop=True,
                                    )
                            et = {}
                            for hl in range(2):
                                et[hl] = expp.tile([128, 1024], BF16, tag="exp", name="et")
                                nc.scalar.activation(et[hl][:], sc[hl][:], EXP)
                            for hl in range(2):
                                hg = 2 * pair + hl
                                lhsT = v_sb[kb][:, hg, :]
                                for half in range(2):
                                    nc.tensor.matmul(
                                        av[(hl, half)][:],
                                        lhsT,
                                        et[hl][:, half * 512:(half + 1) * 512],
                                        start=(kb == 0),
                                        stop=(kb == 15),
                                    )
                        for hl in range(2):
                            hg = 2 * pair + hl
                            for half in range(2):
                                ap = av[(hl, half)]
                                rt = rcpp.tile([128, 512], F32, tag="rcp", name="rt")
                                nc.vector.reciprocal_approx_fast(rt[0:64, :], ap[0:64, :])
                                nc.sync.dma_start(out=rt[64:128, :], in_=rt[0:64, :])
                                q0 = qc * 1024 + half * 512
                                u0 = q0 // 16
                                dst = q2[64:128, :, hg, u0:u0 + 32].transpose([0, 2, 1])
                                nc.vector.tensor_mul(
                                    dst,
                                    ap[64:128, :].rearrange("p (u t) -> p u t", t=16),
                                    rt[64:128, :].rearrange("p (u t) -> p u t", t=16),
                                )
                    # shift-copy the lower 64 partitions for this pair's heads
                    for hl in range(2):
                        hg = 2 * pair + hl
                        nc.sync.dma_start(
                            out=q2[0:64, 1::2, hg, :], in_=q2[64:128, 0::2, hg, :]
                        )

            # ---------------- phase 3: output projection ----------------
            with tc.tile_pool(name="ps3", bufs=1, space="PSUM") as ps3:
                for mb in range(8):
                    ops = ps3.tile([128, 512], F32, tag="o", bufs=2, name="ops")
                    for kt in range(8):
                        nc.tensor.matmul(
                            ops[:],
                            wo[kt][:, mb * 128:(mb + 1) * 128],
                            q2[:, 2 * kt + 1, :, :].rearrange("p a b -> p (a b)"),
                            start=(kt == 0),
                            stop=(kt == 7),
                        )
                    ot = osbp.tile([128, 512], F32, tag="ot", name="ot")
                    nc.vector.tensor_copy(ot[:], ops[:])
                    nc.sync.dma_start(out=out_d[mb], in_=ot[:])
            late_cm.__exit__(None, None, None)

    nc.compile()
    return nc


def _get_program():
    if "nc" not in _CACHE:
        _CACHE["nc"] = _build_program()
    return _CACHE["nc"]


def _make_in_maps(x, Wq, Wk, Wv, Wo):
    import ml_dtypes

    bf16 = ml_dtypes.bfloat16
    wo8 = np.ascontiguousarray(Wo.astype(bf16)).reshape(8, 128, 1024)
    xts = [
        np.ascontiguousarray(x[b].T.astype(np.float16)).reshape(8, 128, 2048)
        for b in range(B)
    ]
    wq16 = Wq.astype(np.float16)
    wk16 = Wk.astype(np.float16)
    wv16 = Wv.astype(np.float16)
    in_maps = []
    for c in range(NCORES):
        b, g = c // 4, c % 4
        cols = slice(4 * g * DH, 4 * (g + 1) * DH)
        in_maps.append(
            {
                "xt": xts[b],
                "wq": np.ascontiguousarray(wq16[:, cols]).reshape(8, 128, 256),
                "wk": np.ascontiguousarray(wk16[:, cols]).reshape(8, 128, 256),
                "wv": np.ascontiguousarray(wv16[:, cols]).reshape(8, 128, 256),
                "wo": wo8,
            }
        )
    return in_maps


def kernel(x, Wq, Wk, Wv, Wo, n_head):
    from concourse.bass_utils import run_bass_kernel_spmd

    assert int(n_head) == H
    x = np.asarray(x, np.float32)
    Wq = np.asarray(Wq, np.float32)
    Wk = np.asarray(Wk, np.float32)
    Wv = np.asarray(Wv, np.float32)
    Wo = np.asarray(Wo, np.float32)

    nc = _get_program()
    in_maps = _make_in_maps(x, Wq, Wk, Wv, Wo)
    res = run_bass_kernel_spmd(nc, in_maps, list(range(NCORES)))

    out = np.empty((B, S, D), np.float32)
    for c in range(NCORES):
        b, g = c // 4, c % 4
        out[b, g * 512:(g + 1) * 512, :] = res.results[c]["out"].reshape(1024, 512).T
    return out


# revision 11
# speedup vs baseline: 1.2027x; 1.1014x over previous
"""Trainium2 Bass kernel for the quirky-reshape 16-head attention layer.

Shapes (hardcoded): x [2, 2048, 1024], Wq/Wk/Wv/Wo [1024, 1024], n_head=16.

Sharding: core c in [0,8) handles batch b=c//4 and head group g=c%4 (heads
4g..4g+3). The reference's quirky `qkv.reshape(b, s, d)` merge makes output
rows [h*128, (h+1)*128) depend only on head h, so each core produces the
disjoint output row block [g*512, (g+1)*512) of its batch — no collectives.

Precision: q/k path (projections + scores) in fp16 (11-bit mantissa), exp /
AV / O-projection in bf16 (fp32-range needed: exp values reach ~1e30), all
matmul accumulation in fp32 PSUM. Measured end-to-end scale-relative absmax
error ~4e-3.

Per-core dataflow:
  qT/kT = W^T @ x^T            [256, 2048] fp16 pair tiles (2 heads x 64)
  V[kb] = per-head [ones | v]  [128, 4, 128] bf16 per 128-key block
  per (head, 1024-wide q chunk):
    for kb in 16: S^T[kb] = kT_h[:,kb].T @ qT_h  -> PSUM [128, 1024] fp32
                  E[kb] = exp(S^T[kb])           (ScalarE -> SBUF bf16)
                  AV[half] += [1|v].T @ E[kb]    -> PSUM [128, 512] x2
    rows 0:64 of AV = softmax denominator (broadcast), rows 64:128 = qkv.
    rcp = reciprocal_approx_fast(denom); DMA rcp to partitions 64:128;
    Q2[64:128, h, q] = qkv * rcp (bf16)
  Q2[0:64, h, 1:] = Q2[64:128, h, :-1]  (DMA shift: O-proj contraction layout)
  out^T[mb] = sum_kt Wo[kt,mb].T @ Q2[:, (h,c) flat][:, (2kt+1)::16]
"""

import numpy as np

B, S, D, H = 2, 2048, 1024, 16
DH = 64
NCORES = 8

_CACHE = {}


def _build_program():
    from concourse import bacc, tile, mybir

    F32 = mybir.dt.float32
    F16 = mybir.dt.float16
    BF16 = mybir.dt.bfloat16
    EXP = mybir.ActivationFunctionType.Exp

    nc = bacc.Bacc(None, target_bir_lowering=False, debug=False)

    xt_d = nc.dram_tensor("xt", [8, 128, 2048], F16, kind="ExternalInput").ap()
    wq_d = nc.dram_tensor("wq", [8, 128, 256], F16, kind="ExternalInput").ap()
    wk_d = nc.dram_tensor("wk", [8, 128, 256], F16, kind="ExternalInput").ap()
    wv_d = nc.dram_tensor("wv", [8, 128, 256], F16, kind="ExternalInput").ap()
    wo_d = nc.dram_tensor("wo", [8, 128, 1024], BF16, kind="ExternalInput").ap()
    out_d = nc.dram_tensor("out", [8, 128, 512], F32, kind="ExternalOutput").ap()

    with tile.TileContext(nc) as tc:
        with (
            tc.tile_pool(name="keep", bufs=1) as keep,
            tc.tile_pool(name="exp", bufs=6) as expp,
            tc.tile_pool(name="rcp", bufs=3) as rcpp,
            tc.tile_pool(name="osb", bufs=2) as osbp,
        ):
            # ---------------- phase 1: projections ----------------
            qk_sb = {}
            v_sb = {}
            with (
                tc.tile_pool(name="inp", bufs=1) as inp,
                tc.tile_pool(name="ps1", bufs=1, space="PSUM") as ps1,
            ):
                xt = []
                wq = []
                wk = []
                wv = []
                for kt in range(8):
                    t = inp.tile([128, 2048], F16, tag=f"xt{kt}", name=f"xt{kt}")
                    nc.sync.dma_start(out=t[:], in_=xt_d[kt])
                    xt.append(t)
                    for nm, lst, dram in (("wq", wq, wq_d), ("wk", wk, wk_d), ("wv", wv, wv_d)):
                        t = inp.tile([128, 256], F16, tag=f"{nm}{kt}", name=f"{nm}{kt}")
                        nc.sync.dma_start(out=t[:], in_=dram[kt])
                        lst.append(t)

                # qT / kT pair tiles [128, 2048]; pair p holds heads (2p, 2p+1)
                for nm, wt in (("q", wq), ("k", wk)):
                    for pair in range(2):
                        dst = keep.tile([128, 2048], F16, tag=f"{nm}T{pair}", name=f"{nm}T{pair}")
                        qk_sb[(nm, pair)] = dst
                        for ch in range(4):
                            ps = ps1.tile([128, 512], F32, tag="qk", bufs=2, name="qkps")
                            for kt in range(8):
                                nc.tensor.matmul(
                                    ps[:],
                                    wt[kt][:, pair * 128:(pair + 1) * 128],
                                    xt[kt][:, ch * 512:(ch + 1) * 512],
                                    start=(kt == 0),
                                    stop=(kt == 7),
                                )
                            nc.vector.tensor_copy(dst[:, ch * 512:(ch + 1) * 512], ps[:])

                # V tiles: [128, 4, 128]; head block hg = [ones(64) | v_hg(64)]
                for kb in range(16):
                    vt = keep.tile([128, 4, 128], BF16, tag=f"v{kb}", name=f"v{kb}")
                    v_sb[kb] = vt
                    nc.vector.memset(vt[:], 1.0)
                    ps = ps1.tile([128, 256], F32, tag="vp", bufs=2, name="vps")
                    for kt in range(8):
                        nc.tensor.matmul(
                            ps[:],
                            xt[kt][:, kb * 128:(kb + 1) * 128],
                            wv[kt][:],
                            start=(kt == 0),
                            stop=(kt == 7),
                        )
                    nc.vector.tensor_copy(vt[:, :, 64:128],
                                          ps[:].rearrange("p (a b) -> p a b", a=4))

            # ---------------- phase 2: attention ----------------
            late_cm = tc.tile_pool(name="late", bufs=1)
            late = late_cm.__enter__()
            q2 = late.tile([128, 16, 4, 128], BF16, tag="q2")
            wo = []
            for kt in range(8):
                t = late.tile([128, 1024], BF16, tag=f"wo{kt}", name=f"wo{kt}")
                nc.sync.dma_start(out=t[:], in_=wo_d[kt])
                wo.append(t)

            with tc.tile_pool(name="ps2", bufs=1, space="PSUM") as ps2:
                for pair in range(2):
                    qT = qk_sb[("q", pair)]
                    kT = qk_sb[("k", pair)]
                    for qc in range(2):
                        av = {}
                        for hl in range(2):
                            for half in range(2):
                                av[(hl, half)] = ps2.tile([128, 512], F32, tag="av", bufs=4, name="av")
                        def av_mms(kbp, et_prev):
                            for hl in range(2):
                                hg = 2 * pair + hl
                                lhsT = v_sb[kbp][:, hg, :]
                                for half in range(2):
                                    nc.tensor.matmul(
                                        av[(hl, half)][:],
                                        lhsT,
                                        et_prev[hl][:, half * 512:(half + 1) * 512],
                                        start=(kbp == 0),
                                        stop=(kbp == 15),
                                    )

                        prev_et = None
                        for kb in range(16):
                            sc = {}
                            for hl in range(2):
                                sc[hl] = ps2.tile([128, 1024], F32, tag="sc", bufs=2, name="sc")
                            # paired scores: head A (rows 0:64) and head B
                            # (rows 64:128) adjacent -> disjoint PE row groups
                            for sub in range(2):
                                q0 = qc * 1024 + sub * 512
                                for hl in range(2):
                                    rows = slice(64 * hl, 64 * hl + 64)
                                    nc.tensor.matmul(
                                        sc[hl][:, sub * 512:(sub + 1) * 512],
                                        kT[rows, kb * 128:(kb + 1) * 128],
                                        qT[rows, q0:q0 + 512],
                                        start=True,
                                        stop=True,
                                    )
                            # software-pipelined AV: consume exp of kb-1 so the
                            # PE never waits on ScalarE inside an iteration
                            if prev_et is not None:
                                av_mms(kb - 1, prev_et)
                            et = {}
                            for hl in range(2):
                                et[hl] = expp.tile([128, 1024], BF16, tag="exp", name="et")
                                nc.scalar.activation(et[hl][:], sc[hl][:], EXP)
                            prev_et = et
                        av_mms(15, prev_et)
                        for hl in range(2):
                            hg = 2 * pair + hl
                            for half in range(2):
                                ap = av[(hl, half)]
                                rt = rcpp.tile([128, 512], F32, tag="rcp", name="rt")
                                nc.vector.reciprocal_approx_fast(rt[0:64, :], ap[0:64, :])
                                nc.sync.dma_start(out=rt[64:128, :], in_=rt[0:64, :])
                                q0 = qc * 1024 + half * 512
                                u0 = q0 // 16
                                dst = q2[64:128, :, hg, u0:u0 + 32].transpose([0, 2, 1])
                                nc.vector.tensor_mul(
                                    dst,
                                    ap[64:128, :].rearrange("p (u t) -> p u t", t=16),
                                    rt[64:128, :].rearrange("p (u t) -> p u t", t=16),
                                )
                    # shift-copy the lower 64 partitions for this pair's heads
                    for hl in range(2):
                        hg = 2 * pair + hl
                        nc.sync.dma_start(
                            out=q2[0:64, 1::2, hg, :], in_=q2[64:128, 0::2, hg, :]
                        )

            # ---------------- phase 3: output projection ----------------
            with tc.tile_pool(name="ps3", bufs=1, space="PSUM") as ps3:
                for mb in range(8):
                    ops = ps3.tile([128, 512], F32, tag="o", bufs=2, name="ops")
                    for kt in range(8):
                        nc.tensor.matmul(
                            ops[:],
                            wo[kt][:, mb * 128:(mb + 1) * 128],
                            q2[:, 2 * kt + 1, :, :].rearrange("p a b -> p (a b)"),
                            start=(kt == 0),
                            stop=(kt == 7),
                        )
                    ot = osbp.tile([128, 512], F32, tag="ot", name="ot")
                    nc.vector.tensor_copy(ot[:], ops[:])
                    nc.sync.dma_start(out=out_d[mb], in_=ot[:])
            late_cm.__exit__(None, None, None)

    nc.compile()
    return nc


def _get_program():
    if "nc" not in _CACHE:
        _CACHE["nc"] = _build_program()
    return _CACHE["nc"]


def _make_in_maps(x, Wq, Wk, Wv, Wo):
    import ml_dtypes

    bf16 = ml_dtypes.bfloat16
    wo8 = np.ascontiguousarray(Wo.astype(bf16)).reshape(8, 128, 1024)
    xts = [
        np.ascontiguousarray(x[b].T.astype(np.float16)).reshape(8, 128, 2048)
        for b in range(B)
    ]
    wq16 = Wq.astype(np.float16)
    wk16 = Wk.astype(np.float16)
    wv16 = Wv.astype(np.float16)
    in_maps = []
    for c in range(NCORES):
        b, g = c // 4, c % 4
        cols = slice(4 * g * DH, 4 * (g + 1) * DH)
        in_maps.append(
            {
                "xt": xts[b],
                "wq": np.ascontiguousarray(wq16[:, cols]).reshape(8, 128, 256),
                "wk": np.ascontiguousarray(wk16[:, cols]).reshape(8, 128, 256),
                "wv": np.ascontiguousarray(wv16[:, cols]).reshape(8, 128, 256),
                "wo": wo8,
            }
        )
    return in_maps


def kernel(x, Wq, Wk, Wv, Wo, n_head):
    from concourse.bass_utils import run_bass_kernel_spmd

    assert int(n_head) == H
    x = np.asarray(x, np.float32)
    Wq = np.asarray(Wq, np.float32)
    Wk = np.asarray(Wk, np.float32)
    Wv = np.asarray(Wv, np.float32)
    Wo = np.asarray(Wo, np.float32)

    nc = _get_program()
    in_maps = _make_in_maps(x, Wq, Wk, Wv, Wo)
    res = run_bass_kernel_spmd(nc, in_maps, list(range(NCORES)))

    out = np.empty((B, S, D), np.float32)
    for c in range(NCORES):
        b, g = c // 4, c % 4
        out[b, g * 512:(g + 1) * 512, :] = res.results[c]["out"].reshape(1024, 512).T
    return out
